# revision 9
# baseline (speedup 1.0000x reference)
"""DGCNN KNN (B=4, N=8192, C=3, K=4) on 8 trn2 NeuronCores.

Strategy (data-parallel, 8 cores = 4 batches x 2 query-halves):
  device (per core, 4096 queries x 8192 candidates):
    s'[q,c] = 2<x_q, x_c> - ||x_c||^2  via one K=14 bf16 PE matmul per
    512-chunk: every f32 input is split into bf16 hi+lo halves and all four
    hi/lo cross products plus the split -||c||^2 term are K-rows, so each
    bf16 product is exact in f32 and the result matches the f32 score to
    f32-accumulation rounding (~5e-5) at 1 cycle/column instead of f32's 4
    (4-way concurrent via tile_position row groups). PSUM -> SBUF via
    ScalarE copy, then per 128-query tile:
    VectorE segmented reduce_max over blocks of 16 -> [128, 512] block
    maxima, max8 + max_index over the block maxima -> top-8 block ids.
    s' differs from the reference pd by the per-row constant -||x_q||^2,
    so ranking is preserved. The 8 best-maximum blocks provably contain
    the true top-8 candidates (the j-th best value's block max ranks
    above all but j-1 other block maxima).
  host: exact f32 rescore of the 8*16=128 surviving candidates per row,
    replicating the reference's operation order, stable (value desc,
    index asc) ordering, take top-4, gather neighbor xyz.
"""

import numpy as np

B, N, C, K = 4, 8192, 3, 4
NCORES = 8
NQ = N // 2  # queries per core
P = 128
CH = 512     # psum bank chunk (f32)
BS = 16      # blockmax block size
KK = 14      # split-bf16 matmul contraction rows
PE_GROUPS = 4

_cache = {}


def _build_kernel(pe_groups=PE_GROUPS):
    import concourse.bacc as bacc
    import concourse.mybir as mybir
    import concourse.tile as tile

    n_tiles = NQ // P
    nblk = N // BS
    nc = bacc.Bacc("TRN2", target_bir_lowering=False, debug=False)

    qT4_d = nc.dram_tensor("qT4", [KK * pe_groups, NQ], mybir.dt.bfloat16, kind="ExternalInput").ap()
    cand_d = nc.dram_tensor("cand", [KK * pe_groups, N], mybir.dt.bfloat16, kind="ExternalInput").ap()
    blk_d = nc.dram_tensor("blk", [NQ, 8], mybir.dt.uint16, kind="ExternalOutput").ap()

    with tile.TileContext(nc) as tc:
        with (
            tc.tile_pool(name="const", bufs=1) as cpool,
            tc.tile_pool(name="work", bufs=3) as wpool,
            tc.tile_pool(name="small", bufs=3) as spool,
            tc.tile_pool(name="ps", bufs=2, space="PSUM") as ppool,
        ):
            # NOTE: keep LoadWeights APs at free-offset 0 (small per-tile
            # tiles) and use only plain 2D DMAs — large free-dim offsets in
            # LDWEIGHTS and partition-strided rearrange DMA views both
            # miscompile (observed garbage past tile 1).
            cand_sb = cpool.tile([32 * (pe_groups - 1) + KK, N], mybir.dt.bfloat16)
            for g in range(pe_groups):
                nc.sync.dma_start(cand_sb[32 * g:32 * g + KK, :], cand_d[KK * g:KK * g + KK, :])

            for t in range(n_tiles):
                lhsT = spool.tile([32 * (pe_groups - 1) + KK, P], mybir.dt.bfloat16, name="lhsT")
                for g in range(pe_groups):
                    nc.sync.dma_start(lhsT[32 * g:32 * g + KK, :], qT4_d[KK * g:KK * g + KK, t * P:(t + 1) * P])
                s_sb = wpool.tile([P, N], mybir.dt.float32, name="s_sb")
                for q4 in range(N // (CH * 4)):
                    pst = ppool.tile([P, CH * 4], mybir.dt.float32, name="pst")
                    for j in range(4):
                        col0 = q4 * CH * 4 + j * CH
                        g = j % pe_groups
                        nc.tensor.matmul(
                            pst[:, j * CH:(j + 1) * CH],
                            lhsT[32 * g:32 * g + KK, :],
                            cand_sb[32 * g:32 * g + KK, col0:col0 + CH],
                            tile_position=(32 * g, 0) if pe_groups > 1 else None,
                        )
                    nc.scalar.copy(s_sb[:, q4 * CH * 4:(q4 + 1) * CH * 4], pst[:])
                bm = spool.tile([P, nblk], mybir.dt.float32, name="bm")
                nc.vector.reduce_max(
                    bm[:],
                    s_sb[:].rearrange("p (b s) -> p b s", s=BS),
                    axis=mybir.AxisListType.X,
                )
                v8 = spool.tile([P, 8], mybir.dt.float32, name="v8")
                i8 = spool.tile([P, 8], mybir.dt.uint16, name="i8")
                nc.vector.max(v8[:], bm[:])
                nc.vector.max_index(i8[:], v8[:], bm[:])
                nc.sync.dma_start(blk_d[t * P:(t + 1) * P, :], i8[:])
    nc.compile()
    return nc


def _get_nc():
    if "nc" not in _cache:
        _cache["nc"] = _build_kernel()
    return _cache["nc"]


def _split_bf16(a):
    import ml_dtypes
    hi = a.astype(ml_dtypes.bfloat16)
    lo = (a - hi.astype(np.float32)).astype(ml_dtypes.bfloat16)
    return hi, lo


def _host_prep(x):
    """x [B,N,3] f32 -> per-core input maps (split-bf16 layout, K=14 rows:
    (qhi x3 | qhi x3 | qlo x3 | qlo x3 | 1 | 1) against
    (2c_hi x3 | 2c_lo x3 | 2c_hi x3 | 2c_lo x3 | -xxc_hi | -xxc_lo))."""
    import ml_dtypes
    bf16 = ml_dtypes.bfloat16
    in_maps = []
    for c in range(NCORES):
        b, h = c // 2, c % 2
        q = x[b, h * NQ:(h + 1) * NQ]
        cd = x[b]
        qhi, qlo = _split_bf16(q)
        chi, clo = _split_bf16(2.0 * cd)
        xxc = (cd[:, 0] * cd[:, 0] + cd[:, 1] * cd[:, 1]) + cd[:, 2] * cd[:, 2]
        xh, xl = _split_bf16(-xxc)
        ones = np.ones(NQ, bf16)
        qT4 = np.stack([qhi[:, 0], qhi[:, 1], qhi[:, 2], qhi[:, 0], qhi[:, 1], qhi[:, 2],
                        qlo[:, 0], qlo[:, 1], qlo[:, 2], qlo[:, 0], qlo[:, 1], qlo[:, 2],
                        ones, ones]).astype(bf16)
        cand = np.stack([chi[:, 0], chi[:, 1], chi[:, 2], clo[:, 0], clo[:, 1], clo[:, 2],
                         chi[:, 0], chi[:, 1], chi[:, 2], clo[:, 0], clo[:, 1], clo[:, 2],
                         xh, xl]).astype(bf16)
        in_maps.append({
            "qT4": np.tile(qT4, (PE_GROUPS, 1)),
            "cand": np.tile(cand, (PE_GROUPS, 1)),
        })
    return in_maps


def _get_runner():
    """Build the bass module once and wrap it in a cached 8-core shard_map jit.

    Mirrors concourse.bass2jax.run_bass_via_pjrt but reuses one jitted
    callable across invocations (run_bass_via_pjrt re-jits per call).
    """
    if "runner" in _cache:
        return _cache["runner"]

    import jax
    import concourse.mybir as mybir
    from jax.sharding import Mesh, PartitionSpec
    from jax.experimental.shard_map import shard_map
    from concourse import bass2jax

    bass2jax.install_neuronx_cc_hook()
    nc = _get_nc()

    partition_name = nc.partition_id_tensor.name if nc.partition_id_tensor else None
    in_names, out_names, out_avals, zero_outs = [], [], [], []
    for alloc in nc.m.functions[0].allocations:
        if not isinstance(alloc, mybir.MemoryLocationSet):
            continue
        name = alloc.memorylocations[0].name
        if alloc.kind == "ExternalInput":
            if name != partition_name:
                in_names.append(name)
        elif alloc.kind == "ExternalOutput":
            shape = tuple(alloc.tensor_shape)
            dtype = mybir.dt.np(alloc.dtype)
            out_names.append(name)
            out_avals.append(jax.core.ShapedArray(shape, dtype))
            zero_outs.append(np.zeros(shape, dtype))
    n_params = len(in_names)
    all_names = in_names + out_names
    if partition_name is not None:
        all_names = all_names + [partition_name]

    def _body(*args):
        operands = list(args)
        if partition_name is not None:
            operands.append(bass2jax.partition_id_tensor())
        outs = bass2jax._bass_exec_p.bind(
            *operands,
            out_avals=tuple(out_avals),
            in_names=tuple(all_names),
            out_names=tuple(out_names),
            lowering_input_output_aliases=(),
            sim_require_finite=True,
            sim_require_nnan=True,
            nc=nc,
        )
        return tuple(outs)

    devices = jax.devices()[:NCORES]
    mesh = Mesh(np.asarray(devices), ("core",))
    n_outs = len(out_names)
    sharded = jax.jit(
        shard_map(
            _body, mesh=mesh,
            in_specs=(PartitionSpec("core"),) * (n_params + n_outs),
            out_specs=(PartitionSpec("core"),) * n_outs,
            check_rep=False,
        ),
        donate_argnums=tuple(range(n_params, n_params + n_outs)),
        keep_unused=True,
    )

    def run(in_maps):
        concat_in = [
            np.concatenate([in_maps[c][nm] for c in range(NCORES)], axis=0)
            for nm in in_names
        ]
        concat_zeros = [
            np.zeros((NCORES * z.shape[0], *z.shape[1:]), z.dtype) for z in zero_outs
        ]
        out_arrs = sharded(*concat_in, *concat_zeros)
        return [
            {nm: np.asarray(out_arrs[i]).reshape(NCORES, *out_avals[i].shape)[c]
             for i, nm in enumerate(out_names)}
            for c in range(NCORES)
        ]

    _cache["runner"] = run
    return run


def run_device(x):
    """Returns blk8 [B, N, 8] int64 (top-8 block ids per point) + results."""
    run = _get_runner()
    in_maps = _host_prep(x)
    results = run(in_maps)
    blk8 = np.empty((B, N, 8), np.int64)
    for c in range(NCORES):
        b, h = c // 2, c % 2
        blk8[b, h * NQ:(h + 1) * NQ] = results[c]["blk"].astype(np.int64)
    return blk8, results


def _host_finish(x, blk8):
    """Exact f32 rescore of 8 blocks x 16 candidates per row, replicating
    the reference's op order; stable top-4; gather."""
    x = np.ascontiguousarray(x, dtype=np.float32)
    bidx = np.arange(B)[:, None, None]
    # candidate ids: [B, N, 8, 16] -> [B, N, 128]
    cidx = (blk8[..., None] * BS + np.arange(BS)).reshape(B, N, 8 * BS)
    c = x[bidx, cidx]                        # [B,N,128,3]
    p0 = x[:, :, None, 0] * c[..., 0]
    p1 = x[:, :, None, 1] * c[..., 1]
    p2 = x[:, :, None, 2] * c[..., 2]
    inner = (p0 + p1) + p2                   # [B,N,128]
    xx = (x[..., 0] * x[..., 0] + x[..., 1] * x[..., 1]) + x[..., 2] * x[..., 2]
    xxc = xx[bidx, cidx]
    pd = (2.0 * inner - xx[:, :, None]) - xxc
    order = np.lexsort((cidx, -pd), axis=-1)[..., :K]
    top4 = np.take_along_axis(cidx, order, axis=-1)   # [B,N,4]
    feature = x[bidx, top4]                  # [B,N,4,3]
    return feature.astype(np.float32)


def kernel(input_data):
    x = np.ascontiguousarray(np.asarray(input_data), dtype=np.float32)
    blk8, _ = run_device(x)
    return _host_finish(x, blk8)
